# revision 26
# baseline (speedup 1.0000x reference)
"""MultiHeadAttention Trainium2 Bass kernel (8 cores), fp16 edition.

Problem: B=2, S=2048, D=1024, H=16 heads, DK=64, fp32 in/out.
  q/k/v = x @ W* + b*; scores = q k^T / 8; attn = softmax; ctx = attn v;
  out = ctx @ Wo + bo.

Sharding (8 cores): batch (2-way) x head-group (4-way tensor parallel).
Core c handles b = c // 4 and heads [4g, 4g+4), g = c % 4 (d' slice of 256).
Each core computes a partial out [S, D]; host sums 4 partials per batch and
adds the bias correction (bv @ Wo + bo).

All device data is fp16 (host converts): halves DMA traffic and, because the
fp16 matmul path has no moving-size restriction (vs fp32r's N>=256), lets the
ctx matmul run "naturally": out[qi, d'] with at as the stationary operand and
v (65 cols: 64 d' + ones column for the softmax denominator) as the moving
operand -- 520 PE rows/step instead of 1024.  The natural [qi, d'] layout
also makes softmax normalization a per-partition broadcast multiply on DVE.
ctx is then transposed via the PE (128-row fp16 transposes) for the output
projection.

Schedule: the exp on ACT (128 x [128,1024], ~1.04us each) is the clock; all
PE work (scores 2 steps ahead, ctx slid ~8 steps behind via buffered at
tiles, and the projection / out-proj matmuls dripped per-kj-tile) hides in
its shadow.  Phases run (c,mt) = (0,0),(1,0),(0,1),(1,1),(2,0),... so the
mt=1 projections are not needed until mid-run.  The ctx slide decays to 0
over the last phases (catch-up) so the tail is short.  Warmup matmuls at
t~0 keep the PE p-state ramped while the input DMAs stream.
"""

import numpy as np

B = 2
S = 2048
D = 1024
H = 16
DK = 64
N_CORES = 8
HL = H // 4  # 4 heads per core
DL = HL * DK  # 256 local d'
QC = 1024  # qi chunk for scores/exp (2 heads x 512)
KJT = S // 128  # 16 kj tiles
KT = D // 128  # 8 contraction tiles for projections
NCH = S // 512  # 4 qi chunks of 512

_CACHED_NC = None


def _build():
    import concourse.bacc as bacc
    import concourse.mybir as mybir
    import concourse.tile as tile
    from concourse.masks import make_identity

    f16 = mybir.dt.float16
    f32 = mybir.dt.float32
    Exp = mybir.ActivationFunctionType.Exp

    nc = bacc.Bacc(None)

    # DRAM params pre-swizzled on host into SBUF layout.
    xT = nc.declare_dram_parameter("xT", [128, KT, S], f16, isOutput=False)
    x0 = nc.declare_dram_parameter("x0", [128, KT, 128], f16, isOutput=False)
    wqh = [
        nc.declare_dram_parameter(f"wq{m}", [128, KT, 128], f16, isOutput=False)
        for m in range(2)
    ]
    wkh = [
        nc.declare_dram_parameter(f"wk{m}", [128, KT, 128], f16, isOutput=False)
        for m in range(2)
    ]
    wvh = [
        nc.declare_dram_parameter(f"wv{m}", [128, KT, 128], f16, isOutput=False)
        for m in range(2)
    ]
    wo = nc.declare_dram_parameter("wo", [128, 2, D], f16, isOutput=False)
    bq = nc.declare_dram_parameter("bq", [128, 2], f32, isOutput=False)
    bk = nc.declare_dram_parameter("bk", [128, 2], f32, isOutput=False)
    out = nc.declare_dram_parameter("out", [S, D], f16, isOutput=True)

    with tile.TileContext(nc) as tc:
        with (
            tc.tile_pool(name="persist", bufs=1) as persist,
            tc.tile_pool(name="atp", bufs=19) as atp,
            tc.tile_pool(name="npl", bufs=2) as npl,
            tc.tile_pool(name="ctxn", bufs=2) as cnp,
            tc.tile_pool(name="ctxT", bufs=2) as ctp,
            tc.tile_pool(name="ob", bufs=2) as obp,
            tc.tile_pool(name="scps", bufs=2, space="PSUM") as scp,
            tc.tile_pool(name="wsps", bufs=2, space="PSUM") as wsp,
            tc.tile_pool(name="cxps", bufs=2, space="PSUM") as cxp,
        ):
            xt = persist.tile([128, KT, S], f16, tag="xt")
            xt0 = persist.tile([128, KT, 128], f16, tag="xt0")
            wqt = [persist.tile([128, KT, 128], f16, tag=f"wqt{m}", name=f"wqt{m}") for m in range(2)]
            wkt = [persist.tile([128, KT, 128], f16, tag=f"wkt{m}", name=f"wkt{m}") for m in range(2)]
            wvt = [persist.tile([128, KT, 128], f16, tag=f"wvt{m}", name=f"wvt{m}") for m in range(2)]
            qT_sb = persist.tile([128, 2, S], f16, tag="qT")
            kT_sb = persist.tile([128, 2, S], f16, tag="kT")
            v_sb = persist.tile([128, KJT, HL, DK + 1], f16, tag="v")
            wo_sb = persist.tile([128, 2, D], f16, tag="wo")
            bq_sb = persist.tile([128, 2], f32, tag="bq")
            bk_sb = persist.tile([128, 2], f32, tag="bk")
            ident = persist.tile([128, 128], f16, tag="ident")
            warm = persist.tile([128, 256], f16, tag="warm")

            nc.vector.memset(warm[:], 0.0)
            nc.vector.memset(v_sb[:, :, :, DK : DK + 1], 1.0)
            make_identity(nc, ident[:])

            # DMA issue order matters: HWDGE serializes issues (625ns each)
            # and DMA_ENGINES is serial; order so the prologue's needs land
            # first.
            nc.sync.dma_start(out=wkt[0][:], in_=wkh[0][:])
            nc.sync.dma_start(out=xt0[:], in_=x0[:])
            nc.sync.dma_start(out=wqt[0][:], in_=wqh[0][:])
            nc.sync.dma_start(out=bk_sb[:], in_=bk[:])
            nc.sync.dma_start(out=bq_sb[:], in_=bq[:])
            nc.sync.dma_start(out=xt[:, :, 128:320], in_=xT[:, :, 128:320])
            nc.sync.dma_start(out=xt[:, :, 320:512], in_=xT[:, :, 320:512])
            nc.sync.dma_start(out=xt[:, :, 512:1024], in_=xT[:, :, 512:1024])
            nc.sync.dma_start(out=xt[:, :, 1024:1536], in_=xT[:, :, 1024:1536])
            nc.sync.dma_start(out=wvt[0][:], in_=wvh[0][:])
            nc.sync.dma_start(out=wvt[1][:], in_=wvh[1][:])
            nc.sync.dma_start(out=xt[:, :, 1536:2048], in_=xT[:, :, 1536:2048])
            nc.sync.dma_start(out=wkt[1][:], in_=wkh[1][:])
            nc.sync.dma_start(out=wqt[1][:], in_=wqh[1][:])
            nc.sync.dma_start(out=wo_sb[:], in_=wo[:])

            # PE warmup: keeps the p-state ramp anchored at t~0 so real
            # matmuls start at full speed.  Garbage into a psum slot that is
            # recycled before the first real ctx needs it.
            wps = cxp.tile([128, 2, 2, DK + 1], f32, tag="cx", name="warm")
            wps_flat = wps[:].rearrange("p a b c -> p (a b c)")

            def warmup(n):
                for _ in range(n):
                    nc.tensor.matmul(
                        wps_flat[0:64, 0:256],
                        warm[:, 0:64],
                        warm[:],
                        start=True,
                        stop=True,
                        skip_group_check=True,
                    )

            def qk_cols(which, mt, c0, c1):
                """Project qT (which=0) / kT (which=1) cols [c0, c1)."""
                wt, dst, bias = (
                    (wqt, qT_sb, bq_sb) if which == 0 else (wkt, kT_sb, bk_sb)
                )
                ps = wsp.tile(
                    [128, c1 - c0], f32, tag="ws", name=f"pj{which}{mt}{c0}"
                )
                for kt in range(KT):
                    mv = xt0[:, kt, :] if c1 <= 128 else xt[:, kt, c0:c1]
                    nc.tensor.matmul(
                        ps[:],
                        wt[mt][:, kt, :],
                        mv,
                        start=(kt == 0),
                        stop=(kt == KT - 1),
                    )
                nc.vector.tensor_scalar_add(
                    out=dst[:, mt, c0:c1], in0=ps[:], scalar1=bias[:, mt : mt + 1]
                )

            def v_half(jt, h2):
                """Project v rows [jt*128, +128) for head pair h2."""
                ps = wsp.tile([128, 128], f32, tag="ws", name=f"vp{jt}{h2}")
                for kt in range(KT):
                    st_ = xt0[:, kt, :] if jt == 0 else xt[:, kt, jt * 128 : (jt + 1) * 128]
                    nc.tensor.matmul(
                        ps[:],
                        st_,
                        wvt[h2][:, kt, :],
                        start=(kt == 0),
                        stop=(kt == KT - 1),
                    )
                nc.vector.tensor_copy(
                    v_sb[:, jt, 2 * h2 : 2 * h2 + 2, 0:DK],
                    ps[:].rearrange("p (h d) -> p h d", h=2),
                )

            ctxn = {}  # c -> [128, 4 qt, 4 h, DK] f16
            ctxT = {}  # c -> [128, 2 dh, 512] f16

            def norm(c, mt, pair, cx):
                """ctx_n = cx[..,:64] / cx[..,64] (broadcast multiply)."""
                rinv = npl.tile([128, 2, 2, 1], f32, tag="rinv")
                nc.vector.reciprocal(out=rinv[:], in_=cx[:, :, :, DK : DK + 1])
                if mt == 0 and pair == 0:
                    ctxn[c] = cnp.tile(
                        [128, 4, 4, DK], f16, tag="ctxn", name=f"ctxn{c}"
                    )
                nc.vector.tensor_mul(
                    ctxn[c][:, pair * 2 : pair * 2 + 2, mt * 2 : mt * 2 + 2, :],
                    cx[:, :, :, 0:DK],
                    rinv[:].broadcast_to([128, 2, 2, DK]),
                )

            def tp(c, dh, qts, drain_act=False):
                """Transpose ctx_n -> ctxT for d'-half dh, qi-tiles qts."""
                if dh == 0 and qts[0] == 0:
                    ctxT[c] = ctp.tile(
                        [128, 2, 512], f16, tag="ctxT", name=f"ctxT{c}"
                    )
                tpps = wsp.tile(
                    [128, 2, 128], f16, tag="ws", name=f"tp{c}{dh}{qts[0]}"
                )
                for j, qt in enumerate(qts):
                    nc.tensor.transpose(
                        tpps[:, j, :],
                        ctxn[c][:, qt, 2 * dh : 2 * dh + 2, :],
                        ident[:],
                    )
                for j, qt in enumerate(qts):
                    dst = ctxT[c][:, dh, qt * 128 : (qt + 1) * 128]
                    if drain_act:
                        # ACT is idle after the last exp; keeps DVE free for
                        # the norm multiplies.
                        nc.scalar.copy(dst, tpps[:, j, :])
                    else:
                        nc.vector.tensor_copy(dst, tpps[:, j, :])

            obt = {}

            def op_piece(c, st, use_pool=False, psum_pool=None):
                """out[c*512+st*128 : +128, :] = ctxT_c[:, :, st]^T @ wo."""
                if st == 0:
                    obt[c] = obp.tile([128, 4, D], f16, tag="ob", name=f"ob{c}")
                pool_ = psum_pool if psum_pool is not None else wsp
                tg = "sc" if pool_ is scp else "ws"
                for nt in range(2):
                    ps = pool_.tile([128, 512], f32, tag=tg, name=f"op{c}{st}{nt}")
                    for dh in range(2):
                        nc.tensor.matmul(
                            ps[:],
                            ctxT[c][:, dh, st * 128 : (st + 1) * 128],
                            wo_sb[:, dh, nt * 512 : (nt + 1) * 512],
                            start=(dh == 0),
                            stop=(dh == 1),
                        )
                    if use_pool and nt == 0:
                        nc.scalar.copy(
                            obt[c][:, st, nt * 512 : (nt + 1) * 512], ps[:]
                        )
                    else:
                        nc.vector.tensor_copy(
                            obt[c][:, st, nt * 512 : (nt + 1) * 512], ps[:]
                        )

            def out_dma(c, st):
                s0 = c * 512 + st * 128
                nc.sync.dma_start(out=out[s0 : s0 + 128, :], in_=obt[c][:, st, :])

            # Phase order: mt=1 projections not needed until step 32.
            PH = [(0, 0), (1, 0), (0, 1), (1, 1), (2, 0), (2, 1), (3, 0), (3, 1)]
            steps = [(c, mt, kj) for (c, mt) in PH for kj in range(KJT)]

            # ctx slide offsets per unit; decays late (catch-up) so the last
            # phase's ctx runs in-step and the tail is short.
            offs = (
                [16] * 48
                + [16] * 4 + [15] * 4 + [14] * 4 + [13] * 4
                + [13] * 4 + [12] * 4 + [11] * 4 + [10] * 4
                + [10] * 4 + [9] * 4 + [8] * 4 + [7] * 4
                + [7] * 4 + [6] * 4 + [5] * 4 + [4] * 4
                + [4] * 2 + [3] * 2 + [2] * 2 + [1] * 2 + [0] * 8
            )
            units_at = {}
            for j in range(128):
                units_at.setdefault(j + offs[j], []).append(j)

            # Dripped background work, emitted between a step's exp and its
            # ctx matmuls.  NOTE: any work feeding sc(i) must be emitted at
            # a step <= i-2 (before emit_sc(i)) or the PE queue deadlocks.
            mid = {}

            def addi(i, th):
                c_, mt_, kj_ = steps[i]
                mid.setdefault((c_, mt_, kj_), []).append(th)

            # kT mt0 kj-tiles 4..15: piece for sc(*,0,kj) dripped at step kj-3
            for kj in range(4, KJT):
                addi(kj - 3, lambda kj=kj: qk_cols(1, 0, kj * 128, (kj + 1) * 128))
            # v[jt] halves at steps jt+8, jt+12 (ctx unit runs at jt+16)
            for jt in range(KJT):
                addi(jt + 8, lambda jt=jt: v_half(jt, 0))
                addi(jt + 12, lambda jt=jt: v_half(jt, 1))
            # qT mt0 n1 (needed by sc emission at step 14): 128-col pieces
            for i, c0 in enumerate(range(512, 1024, 128)):
                addi(7 + 2 * i, lambda c0=c0: qk_cols(0, 0, c0, c0 + 128))
            # qT mt1 n0 (needed at step 30)
            addi(22, lambda: qk_cols(0, 1, 0, 128))
            addi(23, lambda: qk_cols(0, 1, 128, 256))
            addi(25, lambda: qk_cols(0, 1, 256, 512))
            # kT mt1 kj-tiles: piece for sc(*,1,kj) (emitted at step 30+kj)
            for kj in range(KJT):
                addi(28 + kj, lambda kj=kj: qk_cols(1, 1, kj * 128, (kj + 1) * 128))
            # qT mt1 n1 (needed at step 46)
            addi(39, lambda: qk_cols(0, 1, 512, 640))
            addi(41, lambda: qk_cols(0, 1, 640, 768))
            addi(44, lambda: qk_cols(0, 1, 768, 896))
            addi(45, lambda: qk_cols(0, 1, 896, 1024))
            # later qT chunks
            addi(55, lambda: qk_cols(0, 0, 1024, 1280))
            addi(57, lambda: qk_cols(0, 0, 1280, 1536))
            addi(71, lambda: qk_cols(0, 1, 1024, 1280))
            addi(73, lambda: qk_cols(0, 1, 1280, 1536))
            addi(86, lambda: qk_cols(0, 0, 1536, 1792))
            addi(88, lambda: qk_cols(0, 0, 1792, 2048))
            addi(98, lambda: qk_cols(0, 1, 1536, 1792))
            addi(100, lambda: qk_cols(0, 1, 1792, 2048))
            # transposes after the (slid) norms; out-proj + DMA after both
            addi(38, lambda: tp(0, 0, (0, 1)))
            addi(40, lambda: tp(0, 0, (2, 3)))
            addi(53, lambda: tp(1, 0, (0, 1)))
            addi(55, lambda: tp(1, 0, (2, 3)))
            addi(65, lambda: tp(0, 1, (0, 1)))
            addi(67, lambda: tp(0, 1, (2, 3)))
            for st in range(4):
                addi(69 + 3 * st, lambda st=st: op_piece(0, st))
                addi(71 + 3 * st, lambda st=st: out_dma(0, st))
            addi(80, lambda: tp(1, 1, (0, 1)))
            addi(82, lambda: tp(1, 1, (2, 3)))
            for st in range(4):
                addi(84 + 3 * st, lambda st=st: op_piece(1, st))
                addi(86 + 3 * st, lambda st=st: out_dma(1, st))
            addi(95, lambda: tp(2, 0, (0, 1)))
            addi(97, lambda: tp(2, 0, (2, 3)))
            addi(104, lambda: tp(2, 1, (0, 1)))
            addi(106, lambda: tp(2, 1, (2, 3)))
            for st in range(4):
                addi(108 + 2 * st, lambda st=st: op_piece(2, st))
                addi(110 + 2 * st, lambda st=st: out_dma(2, st))
            addi(117, lambda: tp(3, 0, (0, 1)))
            addi(119, lambda: tp(3, 0, (2, 3)))

            sc_t = {}

            def emit_sc(i):
                c, mt, kj = steps[i]
                sc = scp.tile([128, QC], f32, tag="sc", name=f"sc{c}{mt}{kj}")
                col = slice(c * 512, (c + 1) * 512)
                for hp in range(2):
                    hs = slice(64 * hp, 64 * hp + 64)
                    nc.tensor.matmul(
                        sc[:, hp * 512 : (hp + 1) * 512],
                        kT_sb[hs, mt, kj * 128 : (kj + 1) * 128],
                        qT_sb[hs, mt, col],
                        start=True,
                        stop=True,
                    )
                sc_t[i] = sc

            def ctx_pair(c, mt, kj, pair, cx, at_t):
                # start=True zeroes the whole PSUM bank, so only the first
                # matmul touching the bank sets it; the other interleaved
                # accumulation groups run start=False onto the zeroed bank.
                for q2 in range(2):
                    qt = pair * 2 + q2
                    for hp in range(2):
                        nc.tensor.matmul(
                            cx[:, q2, hp, :],
                            at_t[:, hp * 512 + qt * 128 : hp * 512 + (qt + 1) * 128],
                            v_sb[:, kj, 2 * mt + hp, :],
                            start=(kj == 0 and q2 == 0 and hp == 0),
                            stop=(kj == KJT - 1),
                            skip_group_check=True,
                        )

            cxh = {}
            at_tiles = {}

            def ctx_unit(j):
                c, mt, kj = steps[j]
                at_t = at_tiles.pop(j)
                if kj == 0:
                    cxh[(c, mt)] = [
                        cxp.tile(
                            [128, 2, 2, DK + 1], f32, tag="cx", name=f"cx{c}{mt}{p}"
                        )
                        for p in range(2)
                    ]
                if kj < KJT - 1:
                    ctx_pair(c, mt, kj, 0, cxh[(c, mt)][0], at_t)
                    ctx_pair(c, mt, kj, 1, cxh[(c, mt)][1], at_t)
                else:
                    ctx_pair(c, mt, kj, 0, cxh[(c, mt)][0], at_t)
                    norm(c, mt, 0, cxh[(c, mt)][0])
                    ctx_pair(c, mt, kj, 1, cxh[(c, mt)][1], at_t)
                    norm(c, mt, 1, cxh[(c, mt)][1])

            # Prologue: minimum to start sc(0)/sc(1); warmups keep the PE
            # p-state ramped while DMAs stream.
            warmup(10)
            qk_cols(1, 0, 0, 128)
            warmup(4)
            qk_cols(0, 0, 0, 128)
            warmup(6)
            qk_cols(0, 0, 128, 320)
            qk_cols(0, 0, 320, 512)
            emit_sc(0)
            qk_cols(1, 0, 128, 256)
            emit_sc(1)
            qk_cols(1, 0, 256, 512)

            for i, (c, mt, kj) in enumerate(steps):
                at_tiles[i] = atp.tile([128, QC], f16, tag="at", name=f"at{i}")
                nc.scalar.activation(
                    at_tiles[i][:], sc_t.pop(i)[:], Exp, scale=0.125
                )
                if i + 2 < len(steps):
                    emit_sc(i + 2)
                for th in mid.get((c, mt, kj), ()):
                    th()
                for j in units_at.get(i, ()):
                    ctx_unit(j)

            # tail: last chunk's transposes, out-proj, DMAs.  The scores and
            # cx psum pools are free after the last exp/ctx, so spread the 8
            # out-proj pieces over 3 pools (6 slots) to pipeline the drains;
            # ACT (idle now) takes half the drains; DMA out per half-piece.
            tp(NCH - 1, 1, (0, 1), drain_act=True)
            tp(NCH - 1, 1, (2, 3), drain_act=True)
            c3 = NCH - 1
            obt[c3] = obp.tile([128, 4, D], f16, tag="ob", name=f"ob{c3}")
            pools = [wsp, scp, cxp, wsp, scp, cxp, wsp, scp]
            for k in range(8):
                st, nt = k // 2, k % 2
                pool_ = pools[k]
                tg = {id(wsp): "ws", id(scp): "sc", id(cxp): "cx"}[id(pool_)]
                ps = pool_.tile([128, 512], f32, tag=tg, name=f"opt{st}{nt}")
                for dh in range(2):
                    nc.tensor.matmul(
                        ps[:],
                        ctxT[c3][:, dh, st * 128 : (st + 1) * 128],
                        wo_sb[:, dh, nt * 512 : (nt + 1) * 512],
                        start=(dh == 0),
                        stop=(dh == 1),
                    )
                dst = obt[c3][:, st, nt * 512 : (nt + 1) * 512]
                if nt == 0:
                    nc.scalar.copy(dst, ps[:])
                else:
                    nc.vector.tensor_copy(dst, ps[:])
                if st == 3:
                    s0 = c3 * 512 + st * 128
                    nc.sync.dma_start(
                        out=out[s0 : s0 + 128, nt * 512 : (nt + 1) * 512],
                        in_=obt[c3][:, st, nt * 512 : (nt + 1) * 512],
                    )
                elif nt == 1:
                    out_dma(c3, st)

    nc.compile()
    return nc


def _get_nc():
    global _CACHED_NC
    if _CACHED_NC is None:
        _CACHED_NC = _build()
    return _CACHED_NC


def _swz(a, nt):
    """[nt*128, F] -> [128, nt, F] (partition-major) contiguous fp16."""
    F = a.shape[1]
    return np.ascontiguousarray(
        a.reshape(nt, 128, F).transpose(1, 0, 2).astype(np.float16)
    )


def _in_maps(x, Wq, bq, Wk, bk, Wv, bv, Wo, bo):
    xTs = [
        _swz(np.ascontiguousarray(x[b].T).astype(np.float16), KT) for b in range(B)
    ]
    maps = []
    for c in range(N_CORES):
        b, g = c // 4, c % 4
        cs = slice(g * DL, (g + 1) * DL)
        maps.append(
            {
                "xT": xTs[b],
                "x0": np.ascontiguousarray(xTs[b][:, :, 0:128]),
                "wq0": _swz(Wq[:, cs].astype(np.float16)[:, 0:128], KT),
                "wq1": _swz(Wq[:, cs].astype(np.float16)[:, 128:256], KT),
                "wk0": _swz(Wk[:, cs].astype(np.float16)[:, 0:128], KT),
                "wk1": _swz(Wk[:, cs].astype(np.float16)[:, 128:256], KT),
                "wv0": _swz(Wv[:, cs].astype(np.float16)[:, 0:128], KT),
                "wv1": _swz(Wv[:, cs].astype(np.float16)[:, 128:256], KT),
                "wo": _swz(Wo[cs, :].astype(np.float16), 2),
                "bq": np.ascontiguousarray(bq[cs].reshape(2, 128).T),
                "bk": np.ascontiguousarray(bk[cs].reshape(2, 128).T),
            }
        )
    return maps


def _assemble(results, bv, Wo, bo):
    corr = (bv.astype(np.float64) @ Wo.astype(np.float64)) + bo.astype(np.float64)
    outs = []
    for b in range(B):
        acc = np.zeros((S, D), dtype=np.float64)
        for g in range(4):
            acc += results[b * 4 + g]["out"].astype(np.float64)
        outs.append((acc + corr).astype(np.float32))
    return np.stack(outs)


def kernel(x, Wq, bq, Wk, bk, Wv, bv, Wo, bo):
    from concourse.bass_utils import run_bass_kernel_spmd

    x = np.asarray(x, dtype=np.float32)
    Wq = np.asarray(Wq, dtype=np.float32)
    Wk = np.asarray(Wk, dtype=np.float32)
    Wv = np.asarray(Wv, dtype=np.float32)
    Wo = np.asarray(Wo, dtype=np.float32)
    bq = np.asarray(bq, dtype=np.float32)
    bk = np.asarray(bk, dtype=np.float32)
    bv = np.asarray(bv, dtype=np.float32)
    bo = np.asarray(bo, dtype=np.float32)

    nc = _get_nc()
    res = run_bass_kernel_spmd(
        nc, _in_maps(x, Wq, bq, Wk, bk, Wv, bv, Wo, bo), core_ids=list(range(N_CORES))
    )
    return _assemble(res.results, bv, Wo, bo)


# revision 27
# speedup vs baseline: 1.0112x; 1.0112x over previous
"""MultiHeadAttention Trainium2 Bass kernel (8 cores), fp16 edition.

Problem: B=2, S=2048, D=1024, H=16 heads, DK=64, fp32 in/out.
  q/k/v = x @ W* + b*; scores = q k^T / 8; attn = softmax; ctx = attn v;
  out = ctx @ Wo + bo.

Sharding (8 cores): batch (2-way) x head-group (4-way tensor parallel).
Core c handles b = c // 4 and heads [4g, 4g+4), g = c % 4 (d' slice of 256).
Each core computes a partial out [S, D]; host sums 4 partials per batch and
adds the bias correction (bv @ Wo + bo).

All device data is fp16 (host converts): halves DMA traffic and, because the
fp16 matmul path has no moving-size restriction (vs fp32r's N>=256), lets the
ctx matmul run "naturally": out[qi, d'] with at as the stationary operand and
v (65 cols: 64 d' + ones column for the softmax denominator) as the moving
operand -- 520 PE rows/step instead of 1024.  The natural [qi, d'] layout
also makes softmax normalization a per-partition broadcast multiply on DVE.
ctx is then transposed via the PE (128-row fp16 transposes) for the output
projection.

Schedule: the exp on ACT (128 x [128,1024], ~1.04us each) is the clock; all
PE work (scores 2 steps ahead, ctx slid ~8 steps behind via buffered at
tiles, and the projection / out-proj matmuls dripped per-kj-tile) hides in
its shadow.  Phases run (c,mt) = (0,0),(1,0),(0,1),(1,1),(2,0),... so the
mt=1 projections are not needed until mid-run.  The ctx slide decays to 0
over the last phases (catch-up) so the tail is short.  Warmup matmuls at
t~0 keep the PE p-state ramped while the input DMAs stream.
"""

import numpy as np

B = 2
S = 2048
D = 1024
H = 16
DK = 64
N_CORES = 8
HL = H // 4  # 4 heads per core
DL = HL * DK  # 256 local d'
QC = 1024  # qi chunk for scores/exp (2 heads x 512)
KJT = S // 128  # 16 kj tiles
KT = D // 128  # 8 contraction tiles for projections
NCH = S // 512  # 4 qi chunks of 512

_CACHED_NC = None


def _build():
    import concourse.bacc as bacc
    import concourse.mybir as mybir
    import concourse.tile as tile
    from concourse.masks import make_identity

    f16 = mybir.dt.float16
    f32 = mybir.dt.float32
    Exp = mybir.ActivationFunctionType.Exp

    nc = bacc.Bacc(None)

    # DRAM params pre-swizzled on host into SBUF layout.
    xT = nc.declare_dram_parameter("xT", [128, KT, S], f16, isOutput=False)
    x0 = nc.declare_dram_parameter("x0", [128, KT, 128], f16, isOutput=False)
    wqh = [
        nc.declare_dram_parameter(f"wq{m}", [128, KT, 128], f16, isOutput=False)
        for m in range(2)
    ]
    wkh = [
        nc.declare_dram_parameter(f"wk{m}", [128, KT, 128], f16, isOutput=False)
        for m in range(2)
    ]
    wvh = [
        nc.declare_dram_parameter(f"wv{m}", [128, KT, 128], f16, isOutput=False)
        for m in range(2)
    ]
    wo = nc.declare_dram_parameter("wo", [128, 2, D], f16, isOutput=False)
    bq = nc.declare_dram_parameter("bq", [128, 2], f32, isOutput=False)
    bk = nc.declare_dram_parameter("bk", [128, 2], f32, isOutput=False)
    out = nc.declare_dram_parameter("out", [S, D], f16, isOutput=True)

    with tile.TileContext(nc) as tc:
        with (
            tc.tile_pool(name="persist", bufs=1) as persist,
            tc.tile_pool(name="atp", bufs=19) as atp,
            tc.tile_pool(name="npl", bufs=2) as npl,
            tc.tile_pool(name="ctxn", bufs=2) as cnp,
            tc.tile_pool(name="ctxT", bufs=2) as ctp,
            tc.tile_pool(name="ob", bufs=2) as obp,
            tc.tile_pool(name="scps", bufs=2, space="PSUM") as scp,
            tc.tile_pool(name="wsps", bufs=2, space="PSUM") as wsp,
            tc.tile_pool(name="cxps", bufs=2, space="PSUM") as cxp,
        ):
            xt = persist.tile([128, KT, S], f16, tag="xt")
            xt0 = persist.tile([128, KT, 128], f16, tag="xt0")
            wqt = [persist.tile([128, KT, 128], f16, tag=f"wqt{m}", name=f"wqt{m}") for m in range(2)]
            wkt = [persist.tile([128, KT, 128], f16, tag=f"wkt{m}", name=f"wkt{m}") for m in range(2)]
            wvt = [persist.tile([128, KT, 128], f16, tag=f"wvt{m}", name=f"wvt{m}") for m in range(2)]
            qT_sb = persist.tile([128, 2, S], f16, tag="qT")
            kT_sb = persist.tile([128, 2, S], f16, tag="kT")
            v_sb = persist.tile([128, KJT, HL, DK + 1], f16, tag="v")
            wo_sb = persist.tile([128, 2, D], f16, tag="wo")
            bq_sb = persist.tile([128, 2], f32, tag="bq")
            bk_sb = persist.tile([128, 2], f32, tag="bk")
            ident = persist.tile([128, 128], f16, tag="ident")
            warm = persist.tile([128, 256], f16, tag="warm")

            nc.vector.memset(warm[:], 0.0)
            nc.vector.memset(v_sb[:, :, :, DK : DK + 1], 1.0)
            make_identity(nc, ident[:])

            # DMA issue order matters: HWDGE serializes issues (625ns each)
            # and DMA_ENGINES is serial; order so the prologue's needs land
            # first.
            nc.sync.dma_start(out=wkt[0][:], in_=wkh[0][:])
            nc.sync.dma_start(out=xt0[:], in_=x0[:])
            nc.sync.dma_start(out=wqt[0][:], in_=wqh[0][:])
            nc.sync.dma_start(out=bk_sb[:], in_=bk[:])
            nc.sync.dma_start(out=bq_sb[:], in_=bq[:])
            nc.sync.dma_start(out=xt[:, :, 128:512], in_=xT[:, :, 128:512])
            nc.sync.dma_start(out=xt[:, :, 512:1024], in_=xT[:, :, 512:1024])
            nc.sync.dma_start(out=xt[:, :, 1024:1536], in_=xT[:, :, 1024:1536])
            nc.sync.dma_start(out=wvt[0][:], in_=wvh[0][:])
            nc.sync.dma_start(out=wvt[1][:], in_=wvh[1][:])
            nc.sync.dma_start(out=xt[:, :, 1536:2048], in_=xT[:, :, 1536:2048])
            nc.sync.dma_start(out=wkt[1][:], in_=wkh[1][:])
            nc.sync.dma_start(out=wqt[1][:], in_=wqh[1][:])
            nc.sync.dma_start(out=wo_sb[:], in_=wo[:])

            # PE warmup: keeps the p-state ramp anchored at t~0 so real
            # matmuls start at full speed.  Garbage into a psum slot that is
            # recycled before the first real ctx needs it.
            wps = cxp.tile([128, 2, 2, DK + 1], f32, tag="cx", name="warm")
            wps_flat = wps[:].rearrange("p a b c -> p (a b c)")

            def warmup(n):
                for _ in range(n):
                    nc.tensor.matmul(
                        wps_flat[0:64, 0:256],
                        warm[:, 0:64],
                        warm[:],
                        start=True,
                        stop=True,
                        skip_group_check=True,
                    )

            def qk_cols(which, mt, c0, c1):
                """Project qT (which=0) / kT (which=1) cols [c0, c1)."""
                wt, dst, bias = (
                    (wqt, qT_sb, bq_sb) if which == 0 else (wkt, kT_sb, bk_sb)
                )
                ps = wsp.tile(
                    [128, c1 - c0], f32, tag="ws", name=f"pj{which}{mt}{c0}"
                )
                for kt in range(KT):
                    mv = xt0[:, kt, :] if c1 <= 128 else xt[:, kt, c0:c1]
                    nc.tensor.matmul(
                        ps[:],
                        wt[mt][:, kt, :],
                        mv,
                        start=(kt == 0),
                        stop=(kt == KT - 1),
                    )
                nc.vector.tensor_scalar_add(
                    out=dst[:, mt, c0:c1], in0=ps[:], scalar1=bias[:, mt : mt + 1]
                )

            def v_half(jt, h2):
                """Project v rows [jt*128, +128) for head pair h2."""
                ps = wsp.tile([128, 128], f32, tag="ws", name=f"vp{jt}{h2}")
                for kt in range(KT):
                    st_ = xt0[:, kt, :] if jt == 0 else xt[:, kt, jt * 128 : (jt + 1) * 128]
                    nc.tensor.matmul(
                        ps[:],
                        st_,
                        wvt[h2][:, kt, :],
                        start=(kt == 0),
                        stop=(kt == KT - 1),
                    )
                nc.vector.tensor_copy(
                    v_sb[:, jt, 2 * h2 : 2 * h2 + 2, 0:DK],
                    ps[:].rearrange("p (h d) -> p h d", h=2),
                )

            ctxn = {}  # c -> [128, 4 qt, 4 h, DK] f16
            ctxT = {}  # c -> [128, 2 dh, 512] f16

            def norm(c, mt, pair, cx):
                """ctx_n = cx[..,:64] / cx[..,64] (broadcast multiply)."""
                rinv = npl.tile([128, 2, 2, 1], f32, tag="rinv")
                nc.vector.reciprocal(out=rinv[:], in_=cx[:, :, :, DK : DK + 1])
                if mt == 0 and pair == 0:
                    ctxn[c] = cnp.tile(
                        [128, 4, 4, DK], f16, tag="ctxn", name=f"ctxn{c}"
                    )
                nc.vector.tensor_mul(
                    ctxn[c][:, pair * 2 : pair * 2 + 2, mt * 2 : mt * 2 + 2, :],
                    cx[:, :, :, 0:DK],
                    rinv[:].broadcast_to([128, 2, 2, DK]),
                )

            def tp(c, dh, qts, drain_act=False):
                """Transpose ctx_n -> ctxT for d'-half dh, qi-tiles qts."""
                if dh == 0 and qts[0] == 0:
                    ctxT[c] = ctp.tile(
                        [128, 2, 512], f16, tag="ctxT", name=f"ctxT{c}"
                    )
                tpps = wsp.tile(
                    [128, 2, 128], f16, tag="ws", name=f"tp{c}{dh}{qts[0]}"
                )
                for j, qt in enumerate(qts):
                    nc.tensor.transpose(
                        tpps[:, j, :],
                        ctxn[c][:, qt, 2 * dh : 2 * dh + 2, :],
                        ident[:],
                    )
                for j, qt in enumerate(qts):
                    dst = ctxT[c][:, dh, qt * 128 : (qt + 1) * 128]
                    if drain_act:
                        # ACT is idle after the last exp; keeps DVE free for
                        # the norm multiplies.
                        nc.scalar.copy(dst, tpps[:, j, :])
                    else:
                        nc.vector.tensor_copy(dst, tpps[:, j, :])

            obt = {}

            def op_piece(c, st, use_pool=False, psum_pool=None):
                """out[c*512+st*128 : +128, :] = ctxT_c[:, :, st]^T @ wo."""
                if st == 0:
                    obt[c] = obp.tile([128, 4, D], f16, tag="ob", name=f"ob{c}")
                pool_ = psum_pool if psum_pool is not None else wsp
                tg = "sc" if pool_ is scp else "ws"
                for nt in range(2):
                    ps = pool_.tile([128, 512], f32, tag=tg, name=f"op{c}{st}{nt}")
                    for dh in range(2):
                        nc.tensor.matmul(
                            ps[:],
                            ctxT[c][:, dh, st * 128 : (st + 1) * 128],
                            wo_sb[:, dh, nt * 512 : (nt + 1) * 512],
                            start=(dh == 0),
                            stop=(dh == 1),
                        )
                    if use_pool and nt == 0:
                        nc.scalar.copy(
                            obt[c][:, st, nt * 512 : (nt + 1) * 512], ps[:]
                        )
                    else:
                        nc.vector.tensor_copy(
                            obt[c][:, st, nt * 512 : (nt + 1) * 512], ps[:]
                        )

            def out_dma(c, st):
                s0 = c * 512 + st * 128
                nc.sync.dma_start(out=out[s0 : s0 + 128, :], in_=obt[c][:, st, :])

            # Phase order: mt=1 projections not needed until step 32.
            PH = [(0, 0), (1, 0), (0, 1), (1, 1), (2, 0), (2, 1), (3, 0), (3, 1)]
            steps = [(c, mt, kj) for (c, mt) in PH for kj in range(KJT)]

            # ctx slide offsets per unit; decays late (catch-up) so the last
            # phase's ctx runs in-step and the tail is short.
            offs = (
                [16] * 48
                + [16] * 4 + [15] * 4 + [14] * 4 + [13] * 4
                + [13] * 4 + [12] * 4 + [11] * 4 + [10] * 4
                + [10] * 4 + [9] * 4 + [8] * 4 + [7] * 4
                + [7] * 4 + [6] * 4 + [5] * 4 + [4] * 4
                + [4] * 2 + [3] * 2 + [2] * 2 + [1] * 2 + [0] * 8
            )
            units_at = {}
            for j in range(128):
                units_at.setdefault(j + offs[j], []).append(j)

            # Dripped background work, emitted between a step's exp and its
            # ctx matmuls.  NOTE: any work feeding sc(i) must be emitted at
            # a step <= i-2 (before emit_sc(i)) or the PE queue deadlocks.
            mid = {}

            def addi(i, th):
                c_, mt_, kj_ = steps[i]
                mid.setdefault((c_, mt_, kj_), []).append(th)

            # kT mt0 kj-tiles 4..15: piece for sc(*,0,kj) dripped at step kj-3
            for kj in range(4, KJT):
                addi(kj - 3, lambda kj=kj: qk_cols(1, 0, kj * 128, (kj + 1) * 128))
            # v[jt] halves at steps jt+8, jt+12 (ctx unit runs at jt+16)
            for jt in range(KJT):
                addi(jt + 8, lambda jt=jt: v_half(jt, 0))
                addi(jt + 12, lambda jt=jt: v_half(jt, 1))
            # qT mt0 n1 (needed by sc emission at step 14): 128-col pieces
            for i, c0 in enumerate(range(512, 1024, 128)):
                addi(7 + 2 * i, lambda c0=c0: qk_cols(0, 0, c0, c0 + 128))
            # qT mt1 n0 (needed at step 30)
            addi(22, lambda: qk_cols(0, 1, 0, 128))
            addi(23, lambda: qk_cols(0, 1, 128, 256))
            addi(25, lambda: qk_cols(0, 1, 256, 512))
            # kT mt1 kj-tiles: piece for sc(*,1,kj) (emitted at step 30+kj)
            for kj in range(KJT):
                addi(28 + kj, lambda kj=kj: qk_cols(1, 1, kj * 128, (kj + 1) * 128))
            # qT mt1 n1 (needed at step 46)
            addi(39, lambda: qk_cols(0, 1, 512, 640))
            addi(41, lambda: qk_cols(0, 1, 640, 768))
            addi(44, lambda: qk_cols(0, 1, 768, 896))
            addi(45, lambda: qk_cols(0, 1, 896, 1024))
            # later qT chunks
            addi(55, lambda: qk_cols(0, 0, 1024, 1280))
            addi(57, lambda: qk_cols(0, 0, 1280, 1536))
            addi(71, lambda: qk_cols(0, 1, 1024, 1280))
            addi(73, lambda: qk_cols(0, 1, 1280, 1536))
            addi(86, lambda: qk_cols(0, 0, 1536, 1792))
            addi(88, lambda: qk_cols(0, 0, 1792, 2048))
            addi(98, lambda: qk_cols(0, 1, 1536, 1792))
            addi(100, lambda: qk_cols(0, 1, 1792, 2048))
            # transposes after the (slid) norms; out-proj + DMA after both
            addi(38, lambda: tp(0, 0, (0, 1)))
            addi(40, lambda: tp(0, 0, (2, 3)))
            addi(53, lambda: tp(1, 0, (0, 1)))
            addi(55, lambda: tp(1, 0, (2, 3)))
            addi(65, lambda: tp(0, 1, (0, 1)))
            addi(67, lambda: tp(0, 1, (2, 3)))
            for st in range(4):
                addi(69 + 3 * st, lambda st=st: op_piece(0, st))
                addi(71 + 3 * st, lambda st=st: out_dma(0, st))
            addi(80, lambda: tp(1, 1, (0, 1)))
            addi(82, lambda: tp(1, 1, (2, 3)))
            for st in range(4):
                addi(84 + 3 * st, lambda st=st: op_piece(1, st))
                addi(86 + 3 * st, lambda st=st: out_dma(1, st))
            addi(95, lambda: tp(2, 0, (0, 1)))
            addi(97, lambda: tp(2, 0, (2, 3)))
            addi(104, lambda: tp(2, 1, (0, 1)))
            addi(106, lambda: tp(2, 1, (2, 3)))
            for st in range(4):
                addi(108 + 2 * st, lambda st=st: op_piece(2, st))
                addi(110 + 2 * st, lambda st=st: out_dma(2, st))
            addi(117, lambda: tp(3, 0, (0, 1)))
            addi(119, lambda: tp(3, 0, (2, 3)))

            sc_t = {}

            def emit_sc(i):
                c, mt, kj = steps[i]
                sc = scp.tile([128, QC], f32, tag="sc", name=f"sc{c}{mt}{kj}")
                col = slice(c * 512, (c + 1) * 512)
                for hp in range(2):
                    hs = slice(64 * hp, 64 * hp + 64)
                    nc.tensor.matmul(
                        sc[:, hp * 512 : (hp + 1) * 512],
                        kT_sb[hs, mt, kj * 128 : (kj + 1) * 128],
                        qT_sb[hs, mt, col],
                        start=True,
                        stop=True,
                    )
                sc_t[i] = sc

            def ctx_pair(c, mt, kj, pair, cx, at_t):
                # start=True zeroes the whole PSUM bank, so only the first
                # matmul touching the bank sets it; the other interleaved
                # accumulation groups run start=False onto the zeroed bank.
                for q2 in range(2):
                    qt = pair * 2 + q2
                    for hp in range(2):
                        nc.tensor.matmul(
                            cx[:, q2, hp, :],
                            at_t[:, hp * 512 + qt * 128 : hp * 512 + (qt + 1) * 128],
                            v_sb[:, kj, 2 * mt + hp, :],
                            start=(kj == 0 and q2 == 0 and hp == 0),
                            stop=(kj == KJT - 1),
                            skip_group_check=True,
                        )

            cxh = {}
            at_tiles = {}

            def ctx_unit(j):
                c, mt, kj = steps[j]
                at_t = at_tiles.pop(j)
                if kj == 0:
                    cxh[(c, mt)] = [
                        cxp.tile(
                            [128, 2, 2, DK + 1], f32, tag="cx", name=f"cx{c}{mt}{p}"
                        )
                        for p in range(2)
                    ]
                if kj < KJT - 1:
                    ctx_pair(c, mt, kj, 0, cxh[(c, mt)][0], at_t)
                    ctx_pair(c, mt, kj, 1, cxh[(c, mt)][1], at_t)
                else:
                    ctx_pair(c, mt, kj, 0, cxh[(c, mt)][0], at_t)
                    norm(c, mt, 0, cxh[(c, mt)][0])
                    ctx_pair(c, mt, kj, 1, cxh[(c, mt)][1], at_t)
                    norm(c, mt, 1, cxh[(c, mt)][1])

            # Prologue: minimum to start sc(0)/sc(1); warmups keep the PE
            # p-state ramped while DMAs stream.
            warmup(10)
            qk_cols(1, 0, 0, 128)
            warmup(4)
            qk_cols(0, 0, 0, 128)
            warmup(6)
            qk_cols(0, 0, 128, 512)
            emit_sc(0)
            qk_cols(1, 0, 128, 256)
            emit_sc(1)
            qk_cols(1, 0, 256, 512)

            for i, (c, mt, kj) in enumerate(steps):
                at_tiles[i] = atp.tile([128, QC], f16, tag="at", name=f"at{i}")
                nc.scalar.activation(
                    at_tiles[i][:], sc_t.pop(i)[:], Exp, scale=0.125
                )
                if i + 2 < len(steps):
                    emit_sc(i + 2)
                for th in mid.get((c, mt, kj), ()):
                    th()
                for j in units_at.get(i, ()):
                    ctx_unit(j)

            # tail: last chunk's transposes, out-proj, DMAs.  The scores and
            # cx psum pools are free after the last exp/ctx, so spread the 8
            # out-proj pieces over 3 pools (6 slots) to pipeline the drains;
            # ACT (idle now) takes half the drains; DMA out per half-piece.
            tp(NCH - 1, 1, (0, 1), drain_act=True)
            tp(NCH - 1, 1, (2, 3), drain_act=True)
            c3 = NCH - 1
            obt[c3] = obp.tile([128, 4, D], f16, tag="ob", name=f"ob{c3}")
            pools = [wsp, scp, cxp, wsp, scp, cxp, wsp, scp]
            for k in range(8):
                st, nt = k // 2, k % 2
                pool_ = pools[k]
                tg = {id(wsp): "ws", id(scp): "sc", id(cxp): "cx"}[id(pool_)]
                ps = pool_.tile([128, 512], f32, tag=tg, name=f"opt{st}{nt}")
                for dh in range(2):
                    nc.tensor.matmul(
                        ps[:],
                        ctxT[c3][:, dh, st * 128 : (st + 1) * 128],
                        wo_sb[:, dh, nt * 512 : (nt + 1) * 512],
                        start=(dh == 0),
                        stop=(dh == 1),
                    )
                dst = obt[c3][:, st, nt * 512 : (nt + 1) * 512]
                if nt == 0:
                    nc.scalar.copy(dst, ps[:])
                else:
                    nc.vector.tensor_copy(dst, ps[:])
                if st == 3:
                    s0 = c3 * 512 + st * 128
                    nc.sync.dma_start(
                        out=out[s0 : s0 + 128, nt * 512 : (nt + 1) * 512],
                        in_=obt[c3][:, st, nt * 512 : (nt + 1) * 512],
                    )
                elif nt == 1:
                    out_dma(c3, st)

    nc.compile()
    return nc


def _get_nc():
    global _CACHED_NC
    if _CACHED_NC is None:
        _CACHED_NC = _build()
    return _CACHED_NC


def _swz(a, nt):
    """[nt*128, F] -> [128, nt, F] (partition-major) contiguous fp16."""
    F = a.shape[1]
    return np.ascontiguousarray(
        a.reshape(nt, 128, F).transpose(1, 0, 2).astype(np.float16)
    )


def _in_maps(x, Wq, bq, Wk, bk, Wv, bv, Wo, bo):
    xTs = [
        _swz(np.ascontiguousarray(x[b].T).astype(np.float16), KT) for b in range(B)
    ]
    maps = []
    for c in range(N_CORES):
        b, g = c // 4, c % 4
        cs = slice(g * DL, (g + 1) * DL)
        maps.append(
            {
                "xT": xTs[b],
                "x0": np.ascontiguousarray(xTs[b][:, :, 0:128]),
                "wq0": _swz(Wq[:, cs].astype(np.float16)[:, 0:128], KT),
                "wq1": _swz(Wq[:, cs].astype(np.float16)[:, 128:256], KT),
                "wk0": _swz(Wk[:, cs].astype(np.float16)[:, 0:128], KT),
                "wk1": _swz(Wk[:, cs].astype(np.float16)[:, 128:256], KT),
                "wv0": _swz(Wv[:, cs].astype(np.float16)[:, 0:128], KT),
                "wv1": _swz(Wv[:, cs].astype(np.float16)[:, 128:256], KT),
                "wo": _swz(Wo[cs, :].astype(np.float16), 2),
                "bq": np.ascontiguousarray(bq[cs].reshape(2, 128).T),
                "bk": np.ascontiguousarray(bk[cs].reshape(2, 128).T),
            }
        )
    return maps


def _assemble(results, bv, Wo, bo):
    corr = (bv.astype(np.float64) @ Wo.astype(np.float64)) + bo.astype(np.float64)
    outs = []
    for b in range(B):
        acc = np.zeros((S, D), dtype=np.float64)
        for g in range(4):
            acc += results[b * 4 + g]["out"].astype(np.float64)
        outs.append((acc + corr).astype(np.float32))
    return np.stack(outs)


def kernel(x, Wq, bq, Wk, bk, Wv, bv, Wo, bo):
    from concourse.bass_utils import run_bass_kernel_spmd

    x = np.asarray(x, dtype=np.float32)
    Wq = np.asarray(Wq, dtype=np.float32)
    Wk = np.asarray(Wk, dtype=np.float32)
    Wv = np.asarray(Wv, dtype=np.float32)
    Wo = np.asarray(Wo, dtype=np.float32)
    bq = np.asarray(bq, dtype=np.float32)
    bk = np.asarray(bk, dtype=np.float32)
    bv = np.asarray(bv, dtype=np.float32)
    bo = np.asarray(bo, dtype=np.float32)

    nc = _get_nc()
    res = run_bass_kernel_spmd(
        nc, _in_maps(x, Wq, bq, Wk, bk, Wv, bv, Wo, bo), core_ids=list(range(N_CORES))
    )
    return _assemble(res.results, bv, Wo, bo)


# revision 28
# speedup vs baseline: 1.0126x; 1.0014x over previous
"""MultiHeadAttention Trainium2 Bass kernel (8 cores), fp16 edition.

Problem: B=2, S=2048, D=1024, H=16 heads, DK=64, fp32 in/out.
  q/k/v = x @ W* + b*; scores = q k^T / 8; attn = softmax; ctx = attn v;
  out = ctx @ Wo + bo.

Sharding (8 cores): batch (2-way) x head-group (4-way tensor parallel).
Core c handles b = c // 4 and heads [4g, 4g+4), g = c % 4 (d' slice of 256).
Each core computes a partial out [S, D]; host sums 4 partials per batch and
adds the bias correction (bv @ Wo + bo).

All device data is fp16 (host converts): halves DMA traffic and, because the
fp16 matmul path has no moving-size restriction (vs fp32r's N>=256), lets the
ctx matmul run "naturally": out[qi, d'] with at as the stationary operand and
v (65 cols: 64 d' + ones column for the softmax denominator) as the moving
operand -- 520 PE rows/step instead of 1024.  The natural [qi, d'] layout
also makes softmax normalization a per-partition broadcast multiply on DVE.
ctx is then transposed via the PE (128-row fp16 transposes) for the output
projection.

Schedule: the exp on ACT (128 x [128,1024], ~1.04us each) is the clock; all
PE work (scores 2 steps ahead, ctx slid ~8 steps behind via buffered at
tiles, and the projection / out-proj matmuls dripped per-kj-tile) hides in
its shadow.  Phases run (c,mt) = (0,0),(1,0),(0,1),(1,1),(2,0),... so the
mt=1 projections are not needed until mid-run.  The ctx slide decays to 0
over the last phases (catch-up) so the tail is short.  Warmup matmuls at
t~0 keep the PE p-state ramped while the input DMAs stream.
"""

import numpy as np

B = 2
S = 2048
D = 1024
H = 16
DK = 64
N_CORES = 8
HL = H // 4  # 4 heads per core
DL = HL * DK  # 256 local d'
QC = 1024  # qi chunk for scores/exp (2 heads x 512)
KJT = S // 128  # 16 kj tiles
KT = D // 128  # 8 contraction tiles for projections
NCH = S // 512  # 4 qi chunks of 512

_CACHED_NC = None


def _build():
    import concourse.bacc as bacc
    import concourse.mybir as mybir
    import concourse.tile as tile
    from concourse.masks import make_identity

    f16 = mybir.dt.float16
    f32 = mybir.dt.float32
    Exp = mybir.ActivationFunctionType.Exp

    nc = bacc.Bacc(None)

    # DRAM params pre-swizzled on host into SBUF layout.
    xT = nc.declare_dram_parameter("xT", [128, KT, S], f16, isOutput=False)
    x0 = nc.declare_dram_parameter("x0", [128, KT, 128], f16, isOutput=False)
    wqh = [
        nc.declare_dram_parameter(f"wq{m}", [128, KT, 128], f16, isOutput=False)
        for m in range(2)
    ]
    wkh = [
        nc.declare_dram_parameter(f"wk{m}", [128, KT, 128], f16, isOutput=False)
        for m in range(2)
    ]
    wvh = [
        nc.declare_dram_parameter(f"wv{m}", [128, KT, 128], f16, isOutput=False)
        for m in range(2)
    ]
    wo = nc.declare_dram_parameter("wo", [128, 2, D], f16, isOutput=False)
    bq = nc.declare_dram_parameter("bq", [128, 2], f32, isOutput=False)
    bk = nc.declare_dram_parameter("bk", [128, 2], f32, isOutput=False)
    out = nc.declare_dram_parameter("out", [S, D], f16, isOutput=True)

    with tile.TileContext(nc) as tc:
        with (
            tc.tile_pool(name="persist", bufs=1) as persist,
            tc.tile_pool(name="atp", bufs=19) as atp,
            tc.tile_pool(name="npl", bufs=2) as npl,
            tc.tile_pool(name="ctxn", bufs=2) as cnp,
            tc.tile_pool(name="ctxT", bufs=2) as ctp,
            tc.tile_pool(name="ob", bufs=2) as obp,
            tc.tile_pool(name="scps", bufs=2, space="PSUM") as scp,
            tc.tile_pool(name="wsps", bufs=2, space="PSUM") as wsp,
            tc.tile_pool(name="cxps", bufs=2, space="PSUM") as cxp,
        ):
            xt = persist.tile([128, KT, S], f16, tag="xt")
            xt0 = persist.tile([128, KT, 128], f16, tag="xt0")
            wqt = [persist.tile([128, KT, 128], f16, tag=f"wqt{m}", name=f"wqt{m}") for m in range(2)]
            wkt = [persist.tile([128, KT, 128], f16, tag=f"wkt{m}", name=f"wkt{m}") for m in range(2)]
            wvt = [persist.tile([128, KT, 128], f16, tag=f"wvt{m}", name=f"wvt{m}") for m in range(2)]
            qT_sb = persist.tile([128, 2, S], f16, tag="qT")
            kT_sb = persist.tile([128, 2, S], f16, tag="kT")
            v_sb = persist.tile([128, KJT, HL, DK + 1], f16, tag="v")
            wo_sb = persist.tile([128, 2, D], f16, tag="wo")
            bq_sb = persist.tile([128, 2], f32, tag="bq")
            bk_sb = persist.tile([128, 2], f32, tag="bk")
            ident = persist.tile([128, 128], f16, tag="ident")
            warm = persist.tile([128, 256], f16, tag="warm")

            nc.vector.memset(warm[:], 0.0)
            nc.vector.memset(v_sb[:, :, :, DK : DK + 1], 1.0)
            make_identity(nc, ident[:])

            # DMA issue order matters: HWDGE serializes issues (625ns each)
            # and DMA_ENGINES is serial; order so the prologue's needs land
            # first.
            nc.sync.dma_start(out=wkt[0][:], in_=wkh[0][:])
            nc.sync.dma_start(out=xt0[:], in_=x0[:])
            nc.sync.dma_start(out=wqt[0][:], in_=wqh[0][:])
            nc.sync.dma_start(out=bk_sb[:], in_=bk[:])
            nc.sync.dma_start(out=bq_sb[:], in_=bq[:])
            nc.sync.dma_start(out=xt[:, :, 128:512], in_=xT[:, :, 128:512])
            nc.sync.dma_start(out=xt[:, :, 512:1024], in_=xT[:, :, 512:1024])
            nc.sync.dma_start(out=xt[:, :, 1024:1536], in_=xT[:, :, 1024:1536])
            nc.sync.dma_start(out=wvt[0][:], in_=wvh[0][:])
            nc.sync.dma_start(out=wvt[1][:], in_=wvh[1][:])
            nc.sync.dma_start(out=xt[:, :, 1536:2048], in_=xT[:, :, 1536:2048])
            nc.sync.dma_start(out=wkt[1][:], in_=wkh[1][:])
            nc.sync.dma_start(out=wqt[1][:], in_=wqh[1][:])
            nc.sync.dma_start(out=wo_sb[:], in_=wo[:])

            # PE warmup: keeps the p-state ramp anchored at t~0 so real
            # matmuls start at full speed.  Garbage into a psum slot that is
            # recycled before the first real ctx needs it.
            wps = cxp.tile([128, 2, 2, DK + 1], f32, tag="cx", name="warm")
            wps_flat = wps[:].rearrange("p a b c -> p (a b c)")

            def warmup(n):
                for _ in range(n):
                    nc.tensor.matmul(
                        wps_flat[0:64, 0:256],
                        warm[:, 0:64],
                        warm[:],
                        start=True,
                        stop=True,
                        skip_group_check=True,
                    )

            def qk_cols(which, mt, c0, c1):
                """Project qT (which=0) / kT (which=1) cols [c0, c1)."""
                wt, dst, bias = (
                    (wqt, qT_sb, bq_sb) if which == 0 else (wkt, kT_sb, bk_sb)
                )
                ps = wsp.tile(
                    [128, c1 - c0], f32, tag="ws", name=f"pj{which}{mt}{c0}"
                )
                for kt in range(KT):
                    mv = xt0[:, kt, :] if c1 <= 128 else xt[:, kt, c0:c1]
                    nc.tensor.matmul(
                        ps[:],
                        wt[mt][:, kt, :],
                        mv,
                        start=(kt == 0),
                        stop=(kt == KT - 1),
                    )
                nc.vector.tensor_scalar_add(
                    out=dst[:, mt, c0:c1], in0=ps[:], scalar1=bias[:, mt : mt + 1]
                )

            def v_half(jt, h2):
                """Project v rows [jt*128, +128) for head pair h2."""
                ps = wsp.tile([128, 128], f32, tag="ws", name=f"vp{jt}{h2}")
                for kt in range(KT):
                    st_ = xt0[:, kt, :] if jt == 0 else xt[:, kt, jt * 128 : (jt + 1) * 128]
                    nc.tensor.matmul(
                        ps[:],
                        st_,
                        wvt[h2][:, kt, :],
                        start=(kt == 0),
                        stop=(kt == KT - 1),
                    )
                nc.vector.tensor_copy(
                    v_sb[:, jt, 2 * h2 : 2 * h2 + 2, 0:DK],
                    ps[:].rearrange("p (h d) -> p h d", h=2),
                )

            ctxn = {}  # c -> [128, 4 qt, 4 h, DK] f16
            ctxT = {}  # c -> [128, 2 dh, 512] f16

            def norm(c, mt, pair, cx):
                """ctx_n = cx[..,:64] / cx[..,64] (broadcast multiply)."""
                rinv = npl.tile([128, 2, 2, 1], f32, tag="rinv")
                nc.vector.reciprocal(out=rinv[:], in_=cx[:, :, :, DK : DK + 1])
                if mt == 0 and pair == 0:
                    ctxn[c] = cnp.tile(
                        [128, 4, 4, DK], f16, tag="ctxn", name=f"ctxn{c}"
                    )
                nc.vector.tensor_mul(
                    ctxn[c][:, pair * 2 : pair * 2 + 2, mt * 2 : mt * 2 + 2, :],
                    cx[:, :, :, 0:DK],
                    rinv[:].broadcast_to([128, 2, 2, DK]),
                )

            def tp(c, dh, qts, drain_act=False):
                """Transpose ctx_n -> ctxT for d'-half dh, qi-tiles qts."""
                if dh == 0 and qts[0] == 0:
                    ctxT[c] = ctp.tile(
                        [128, 2, 512], f16, tag="ctxT", name=f"ctxT{c}"
                    )
                tpps = wsp.tile(
                    [128, 2, 128], f16, tag="ws", name=f"tp{c}{dh}{qts[0]}"
                )
                for j, qt in enumerate(qts):
                    nc.tensor.transpose(
                        tpps[:, j, :],
                        ctxn[c][:, qt, 2 * dh : 2 * dh + 2, :],
                        ident[:],
                    )
                for j, qt in enumerate(qts):
                    dst = ctxT[c][:, dh, qt * 128 : (qt + 1) * 128]
                    if drain_act:
                        # ACT is idle after the last exp; keeps DVE free for
                        # the norm multiplies.
                        nc.scalar.copy(dst, tpps[:, j, :])
                    else:
                        nc.vector.tensor_copy(dst, tpps[:, j, :])

            obt = {}

            def op_piece(c, st, use_pool=False, psum_pool=None):
                """out[c*512+st*128 : +128, :] = ctxT_c[:, :, st]^T @ wo."""
                if st == 0:
                    obt[c] = obp.tile([128, 4, D], f16, tag="ob", name=f"ob{c}")
                pool_ = psum_pool if psum_pool is not None else wsp
                tg = "sc" if pool_ is scp else "ws"
                for nt in range(2):
                    ps = pool_.tile([128, 512], f32, tag=tg, name=f"op{c}{st}{nt}")
                    for dh in range(2):
                        nc.tensor.matmul(
                            ps[:],
                            ctxT[c][:, dh, st * 128 : (st + 1) * 128],
                            wo_sb[:, dh, nt * 512 : (nt + 1) * 512],
                            start=(dh == 0),
                            stop=(dh == 1),
                        )
                    if use_pool and nt == 0:
                        nc.scalar.copy(
                            obt[c][:, st, nt * 512 : (nt + 1) * 512], ps[:]
                        )
                    else:
                        nc.vector.tensor_copy(
                            obt[c][:, st, nt * 512 : (nt + 1) * 512], ps[:]
                        )

            def out_dma(c, st):
                s0 = c * 512 + st * 128
                nc.sync.dma_start(out=out[s0 : s0 + 128, :], in_=obt[c][:, st, :])

            # Phase order: mt=1 projections not needed until step 32.
            PH = [(0, 0), (1, 0), (0, 1), (1, 1), (2, 0), (2, 1), (3, 0), (3, 1)]
            steps = [(c, mt, kj) for (c, mt) in PH for kj in range(KJT)]

            # ctx slide offsets per unit; decays late (catch-up) so the last
            # phase's ctx runs in-step and the tail is short.
            offs = (
                [16] * 48
                + [16] * 4 + [15] * 4 + [14] * 4 + [13] * 4
                + [13] * 4 + [12] * 4 + [11] * 4 + [10] * 4
                + [10] * 4 + [9] * 4 + [8] * 4 + [7] * 4
                + [7] * 4 + [6] * 4 + [5] * 4 + [4] * 4
                + [4] * 2 + [3] * 2 + [2] * 2 + [1] * 2 + [0] * 8
            )
            units_at = {}
            for j in range(128):
                units_at.setdefault(j + offs[j], []).append(j)

            # Dripped background work, emitted between a step's exp and its
            # ctx matmuls.  NOTE: any work feeding sc(i) must be emitted at
            # a step <= i-2 (before emit_sc(i)) or the PE queue deadlocks.
            mid = {}

            def addi(i, th):
                c_, mt_, kj_ = steps[i]
                mid.setdefault((c_, mt_, kj_), []).append(th)

            # kT mt0 kj-tiles 4..15: piece for sc(*,0,kj) dripped at step kj-3
            for kj in range(4, KJT):
                addi(kj - 3, lambda kj=kj: qk_cols(1, 0, kj * 128, (kj + 1) * 128))
            # v[jt] halves at steps jt+8, jt+12 (ctx unit runs at jt+16)
            for jt in range(KJT):
                addi(jt + 8, lambda jt=jt: v_half(jt, 0))
                addi(jt + 12, lambda jt=jt: v_half(jt, 1))
            # qT mt0 n1 (needed by sc emission at step 14): 128-col pieces
            for i, c0 in enumerate(range(512, 1024, 128)):
                addi(7 + 2 * i, lambda c0=c0: qk_cols(0, 0, c0, c0 + 128))
            # qT mt1 n0 (needed at step 30)
            addi(22, lambda: qk_cols(0, 1, 0, 128))
            addi(23, lambda: qk_cols(0, 1, 128, 256))
            addi(25, lambda: qk_cols(0, 1, 256, 512))
            # kT mt1 kj-tiles: piece for sc(*,1,kj) (emitted at step 30+kj)
            for kj in range(KJT):
                addi(28 + kj, lambda kj=kj: qk_cols(1, 1, kj * 128, (kj + 1) * 128))
            # qT mt1 n1 (needed at step 46)
            addi(39, lambda: qk_cols(0, 1, 512, 640))
            addi(41, lambda: qk_cols(0, 1, 640, 768))
            addi(44, lambda: qk_cols(0, 1, 768, 896))
            addi(45, lambda: qk_cols(0, 1, 896, 1024))
            # later qT chunks
            addi(55, lambda: qk_cols(0, 0, 1024, 1280))
            addi(57, lambda: qk_cols(0, 0, 1280, 1536))
            addi(71, lambda: qk_cols(0, 1, 1024, 1280))
            addi(73, lambda: qk_cols(0, 1, 1280, 1536))
            addi(86, lambda: qk_cols(0, 0, 1536, 1792))
            addi(88, lambda: qk_cols(0, 0, 1792, 2048))
            addi(98, lambda: qk_cols(0, 1, 1536, 1792))
            addi(100, lambda: qk_cols(0, 1, 1792, 2048))
            # transposes after the (slid) norms; out-proj + DMA after both
            addi(38, lambda: tp(0, 0, (0, 1)))
            addi(40, lambda: tp(0, 0, (2, 3)))
            addi(53, lambda: tp(1, 0, (0, 1)))
            addi(55, lambda: tp(1, 0, (2, 3)))
            addi(65, lambda: tp(0, 1, (0, 1)))
            addi(67, lambda: tp(0, 1, (2, 3)))
            for st in range(4):
                addi(69 + 3 * st, lambda st=st: op_piece(0, st))
                addi(71 + 3 * st, lambda st=st: out_dma(0, st))
            addi(80, lambda: tp(1, 1, (0, 1)))
            addi(82, lambda: tp(1, 1, (2, 3)))
            for st in range(4):
                addi(84 + 3 * st, lambda st=st: op_piece(1, st))
                addi(86 + 3 * st, lambda st=st: out_dma(1, st))
            addi(95, lambda: tp(2, 0, (0, 1)))
            addi(97, lambda: tp(2, 0, (2, 3)))
            addi(104, lambda: tp(2, 1, (0, 1)))
            addi(106, lambda: tp(2, 1, (2, 3)))
            for st in range(4):
                addi(108 + 2 * st, lambda st=st: op_piece(2, st))
                addi(110 + 2 * st, lambda st=st: out_dma(2, st))
            addi(117, lambda: tp(3, 0, (0, 1)))
            addi(119, lambda: tp(3, 0, (2, 3)))

            sc_t = {}

            def emit_sc(i):
                c, mt, kj = steps[i]
                sc = scp.tile([128, QC], f32, tag="sc", name=f"sc{c}{mt}{kj}")
                col = slice(c * 512, (c + 1) * 512)
                for hp in range(2):
                    hs = slice(64 * hp, 64 * hp + 64)
                    nc.tensor.matmul(
                        sc[:, hp * 512 : (hp + 1) * 512],
                        kT_sb[hs, mt, kj * 128 : (kj + 1) * 128],
                        qT_sb[hs, mt, col],
                        start=True,
                        stop=True,
                    )
                sc_t[i] = sc

            def ctx_pair(c, mt, kj, pair, cx, at_t):
                # start=True zeroes the whole PSUM bank, so only the first
                # matmul touching the bank sets it; the other interleaved
                # accumulation groups run start=False onto the zeroed bank.
                for q2 in range(2):
                    qt = pair * 2 + q2
                    for hp in range(2):
                        nc.tensor.matmul(
                            cx[:, q2, hp, :],
                            at_t[:, hp * 512 + qt * 128 : hp * 512 + (qt + 1) * 128],
                            v_sb[:, kj, 2 * mt + hp, :],
                            start=(kj == 0 and q2 == 0 and hp == 0),
                            stop=(kj == KJT - 1),
                            skip_group_check=True,
                        )

            cxh = {}
            at_tiles = {}

            def ctx_unit(j):
                c, mt, kj = steps[j]
                at_t = at_tiles.pop(j)
                if kj == 0:
                    cxh[(c, mt)] = [
                        cxp.tile(
                            [128, 2, 2, DK + 1], f32, tag="cx", name=f"cx{c}{mt}{p}"
                        )
                        for p in range(2)
                    ]
                if kj < KJT - 1:
                    ctx_pair(c, mt, kj, 0, cxh[(c, mt)][0], at_t)
                    ctx_pair(c, mt, kj, 1, cxh[(c, mt)][1], at_t)
                else:
                    ctx_pair(c, mt, kj, 0, cxh[(c, mt)][0], at_t)
                    norm(c, mt, 0, cxh[(c, mt)][0])
                    ctx_pair(c, mt, kj, 1, cxh[(c, mt)][1], at_t)
                    norm(c, mt, 1, cxh[(c, mt)][1])

            # Prologue: minimum to start sc(0)/sc(1); warmups keep the PE
            # p-state ramped while DMAs stream.
            warmup(10)
            qk_cols(1, 0, 0, 128)
            warmup(4)
            qk_cols(0, 0, 0, 128)
            warmup(6)
            qk_cols(0, 0, 128, 512)
            emit_sc(0)
            qk_cols(1, 0, 128, 256)
            emit_sc(1)
            qk_cols(1, 0, 256, 512)

            for i, (c, mt, kj) in enumerate(steps):
                at_tiles[i] = atp.tile([128, QC], f16, tag="at", name=f"at{i}")
                nc.scalar.activation(
                    at_tiles[i][:], sc_t.pop(i)[:], Exp, scale=0.125
                )
                if i + 2 < len(steps):
                    emit_sc(i + 2)
                for th in mid.get((c, mt, kj), ()):
                    th()
                for j in units_at.get(i, ()):
                    ctx_unit(j)

            # tail: last chunk's transposes, out-proj, DMAs.  The scores and
            # cx psum pools are free after the last exp/ctx, so spread the 8
            # out-proj pieces over 3 pools (6 slots) to pipeline the drains;
            # ACT (idle now) takes half the drains; DMA out per half-piece.
            tp(NCH - 1, 1, (0, 1), drain_act=True)
            tp(NCH - 1, 1, (2, 3), drain_act=True)
            c3 = NCH - 1
            obt[c3] = obp.tile([128, 4, D], f16, tag="ob", name=f"ob{c3}")
            pools = [wsp, scp, cxp, wsp, scp, cxp, wsp, scp]
            for k in range(8):
                st, nt = k // 2, k % 2
                pool_ = pools[k]
                tg = {id(wsp): "ws", id(scp): "sc", id(cxp): "cx"}[id(pool_)]
                ps = pool_.tile([128, 512], f32, tag=tg, name=f"opt{st}{nt}")
                for dh in range(2):
                    nc.tensor.matmul(
                        ps[:],
                        ctxT[c3][:, dh, st * 128 : (st + 1) * 128],
                        wo_sb[:, dh, nt * 512 : (nt + 1) * 512],
                        start=(dh == 0),
                        stop=(dh == 1),
                    )
                dst = obt[c3][:, st, nt * 512 : (nt + 1) * 512]
                if nt == 0:
                    nc.scalar.copy(dst, ps[:])
                else:
                    nc.vector.tensor_copy(dst, ps[:])
                if nt == 1:
                    out_dma(c3, st)

    nc.compile()
    return nc


def _get_nc():
    global _CACHED_NC
    if _CACHED_NC is None:
        _CACHED_NC = _build()
    return _CACHED_NC


def _swz(a, nt):
    """[nt*128, F] -> [128, nt, F] (partition-major) contiguous fp16."""
    F = a.shape[1]
    return np.ascontiguousarray(
        a.reshape(nt, 128, F).transpose(1, 0, 2).astype(np.float16)
    )


def _in_maps(x, Wq, bq, Wk, bk, Wv, bv, Wo, bo):
    xTs = [
        _swz(np.ascontiguousarray(x[b].T).astype(np.float16), KT) for b in range(B)
    ]
    maps = []
    for c in range(N_CORES):
        b, g = c // 4, c % 4
        cs = slice(g * DL, (g + 1) * DL)
        maps.append(
            {
                "xT": xTs[b],
                "x0": np.ascontiguousarray(xTs[b][:, :, 0:128]),
                "wq0": _swz(Wq[:, cs].astype(np.float16)[:, 0:128], KT),
                "wq1": _swz(Wq[:, cs].astype(np.float16)[:, 128:256], KT),
                "wk0": _swz(Wk[:, cs].astype(np.float16)[:, 0:128], KT),
                "wk1": _swz(Wk[:, cs].astype(np.float16)[:, 128:256], KT),
                "wv0": _swz(Wv[:, cs].astype(np.float16)[:, 0:128], KT),
                "wv1": _swz(Wv[:, cs].astype(np.float16)[:, 128:256], KT),
                "wo": _swz(Wo[cs, :].astype(np.float16), 2),
                "bq": np.ascontiguousarray(bq[cs].reshape(2, 128).T),
                "bk": np.ascontiguousarray(bk[cs].reshape(2, 128).T),
            }
        )
    return maps


def _assemble(results, bv, Wo, bo):
    corr = (bv.astype(np.float64) @ Wo.astype(np.float64)) + bo.astype(np.float64)
    outs = []
    for b in range(B):
        acc = np.zeros((S, D), dtype=np.float64)
        for g in range(4):
            acc += results[b * 4 + g]["out"].astype(np.float64)
        outs.append((acc + corr).astype(np.float32))
    return np.stack(outs)


def kernel(x, Wq, bq, Wk, bk, Wv, bv, Wo, bo):
    from concourse.bass_utils import run_bass_kernel_spmd

    x = np.asarray(x, dtype=np.float32)
    Wq = np.asarray(Wq, dtype=np.float32)
    Wk = np.asarray(Wk, dtype=np.float32)
    Wv = np.asarray(Wv, dtype=np.float32)
    Wo = np.asarray(Wo, dtype=np.float32)
    bq = np.asarray(bq, dtype=np.float32)
    bk = np.asarray(bk, dtype=np.float32)
    bv = np.asarray(bv, dtype=np.float32)
    bo = np.asarray(bo, dtype=np.float32)

    nc = _get_nc()
    res = run_bass_kernel_spmd(
        nc, _in_maps(x, Wq, bq, Wk, bk, Wv, bv, Wo, bo), core_ids=list(range(N_CORES))
    )
    return _assemble(res.results, bv, Wo, bo)
